# revision 1
# baseline (speedup 1.0000x reference)
"""Trainium2 Bass kernel for nn_ConvAttention (retrieval_knn).

Data-parallel over batch B=32 across 8 NeuronCores (4 batches/core).

v2 design notes (vs baseline):
- All conv matmuls in fp8e4m3 with MatmulPerfMode.DoubleRow (2 k-tiles of
  128 per pass) where contraction >= 256; weights host-prescaled x16 and
  descaled in the PSUM-evacuating activation (scale=1/16).
- Weight-stationary over batch pairs to amortize LDWEIGHTS, and dense
  back-to-back matmul streams to keep the PE HAM clock gate at 8/8
  (2.4 GHz) instead of the cold 4/8 (1.2 GHz) the baseline ran at.
- Scores via one matmul with augmented contraction row 80:
  qaug row80 = ones, kaug row80 = -0.5*|k|^2; the -0.5*|q|^2 term enters
  as the exp() per-partition bias via a 1-column matmul into the same
  PSUM bank (cols 400) => pss = qk - 0.5k2, bias = -0.0005*q2.
- Elementwise pipeline in bf16 spread across ACT (exp, ln), DVE
  (psum evacs, masked-accumulate, tinies), Pool (u-mul, attn scale,
  squares) -- Pool cannot touch PSUM.
- prior shipped as bf16 (prior+1e-8), keys/queries as fp8, outputs
  returned as bf16 and upcast on host. Mask folded in as a bf16 row
  broadcast.
- PSUM pair tiles are [128, 2, 512] f32 (bank-aligned halves) so one
  DVE/ACT op evacuates two batches' conv outputs.
"""

import numpy as np
import ml_dtypes

B, N_MEL, N_TEXT, N_ATT, T1, T2 = 32, 80, 512, 80, 1600, 400
N_CORES = 8
B_LOC = B // N_CORES  # 4

_CACHE = {}

BF16 = ml_dtypes.bfloat16
FP8 = ml_dtypes.float8_e4m3
WSCALE = 16.0

# t1 groups: (o1, n_tiles, tp_of_each[last may be 64])
GROUPS = [(0, 4, 128), (512, 4, 128), (1024, 4, 128), (1536, 1, 64)]


# no_dr_q1: qconv1 DR (80-row k-tiles + mixed group) faults the PE
# attn_dve: gpsimd TensorScalarPtr w/ AP scalar measured 5.9us/op -> DVE
# um_stt: tensor_tensor_reduce faults the PE at runtime; use stt
OPTS = {"no_dr_q1", "attn_dve", "um_stt"}


def _build_program():
    opts = OPTS
    import concourse.bacc as bacc
    import concourse.tile as tile
    import concourse.mybir as mybir
    import concourse.bass as bass

    f32 = mybir.dt.float32
    bf16 = mybir.dt.bfloat16
    fp8 = mybir.dt.float8e4
    AF = mybir.ActivationFunctionType
    ALU = mybir.AluOpType
    DR = mybir.MatmulPerfMode.DoubleRow

    nc = bacc.Bacc(None, target_bir_lowering=False)

    # ---- DRAM parameters (per-core shapes) ----
    keys8_d = nc.declare_dram_parameter("keys8", [B_LOC, N_TEXT, T2], fp8, isOutput=False)
    qrs8_d = nc.declare_dram_parameter("qrs8", [B_LOC, N_MEL, T1], fp8, isOutput=False)
    pm_d = nc.declare_dram_parameter("pm", [B_LOC, T1, T2], bf16, isOutput=False)
    mrowf_d = nc.declare_dram_parameter("mrowf", [B_LOC, T2], bf16, isOutput=False)
    ones1_d = nc.declare_dram_parameter("ones1", [1, B_LOC * T1], bf16, isOutput=False)
    zeros1_d = nc.declare_dram_parameter("zeros1", [1, B_LOC * T1], bf16, isOutput=False)
    NAUG = 97  # rows 0:80 data, 80:96 zeros, 96 augmented (32-aligned base)

    kw1_d = nc.declare_dram_parameter("kw1p", [128, 8, 6, 2, 128], fp8, isOutput=False)
    kb1n_d = nc.declare_dram_parameter("kb1n", [128, 8], f32, isOutput=False)
    kb1p_d = nc.declare_dram_parameter("kb1p", [128, 8], f32, isOutput=False)
    kw2_d = nc.declare_dram_parameter("kw2p", [128, 4, 2, N_ATT], fp8, isOutput=False)
    kb2_d = nc.declare_dram_parameter("kb2c", [N_ATT, 1], f32, isOutput=False)
    qw1adr_d = nc.declare_dram_parameter("qw1adr", [N_MEL, 2, 128], fp8, isOutput=False)
    qw1as_d = nc.declare_dram_parameter("qw1as", [N_MEL, 128], fp8, isOutput=False)
    qw1bdr_d = nc.declare_dram_parameter("qw1bdr", [N_MEL, 2, 32], fp8, isOutput=False)
    qw1bs_d = nc.declare_dram_parameter("qw1bs", [N_MEL, 32], fp8, isOutput=False)
    qb1an_d = nc.declare_dram_parameter("qb1an", [128, 1], f32, isOutput=False)
    qb1ap_d = nc.declare_dram_parameter("qb1ap", [128, 1], f32, isOutput=False)
    qb1bn_d = nc.declare_dram_parameter("qb1bn", [32, 1], f32, isOutput=False)
    qb1bp_d = nc.declare_dram_parameter("qb1bp", [32, 1], f32, isOutput=False)
    qw2a_d = nc.declare_dram_parameter("qw2a", [128, N_MEL], fp8, isOutput=False)
    qw2b_d = nc.declare_dram_parameter("qw2b", [32, N_MEL], fp8, isOutput=False)
    qb2_d = nc.declare_dram_parameter("qb2c", [N_MEL, 1], f32, isOutput=False)
    qw3_d = nc.declare_dram_parameter("qw3T", [N_MEL, N_ATT], fp8, isOutput=False)
    qb3_d = nc.declare_dram_parameter("qb3c", [N_ATT, 1], f32, isOutput=False)

    attn_d = nc.declare_dram_parameter("attn", [B_LOC, T1, T2], bf16, isOutput=True)
    logp_d = nc.declare_dram_parameter("logp", [B_LOC, T1, T2], bf16, isOutput=True)

    def reap(base_ap, dims):
        """AP at base_ap's offset with base partition entry + custom free dims."""
        return bass.AP(tensor=base_ap.tensor, offset=base_ap.offset,
                       ap=[list(base_ap.ap[0])] + [list(d) for d in dims])

    def dram_ap(base_ap, extra_off, dims):
        return bass.AP(tensor=base_ap.tensor, offset=base_ap.offset + extra_off,
                       ap=[list(d) for d in dims])

    def apj(a, j):
        """Slice dim1 (size 2) of a 3D AP [part, [s,2], [s2,N]] at index j."""
        part, d1, d2 = a.ap
        return bass.AP(tensor=a.tensor, offset=a.offset + j * d1[0],
                       ap=[list(part), list(d2)])

    def mm_dr(out, w3, r3, start, stop, site=""):
        if "no_dr" in opts or f"no_dr_{site}" in opts:
            nc.tensor.matmul(out, apj(w3, 0), apj(r3, 0), start=start, stop=False)
            nc.tensor.matmul(out, apj(w3, 1), apj(r3, 1), start=False, stop=stop)
        else:
            nc.tensor.matmul(out, w3, r3, start=start, stop=stop,
                             perf_mode=mybir.MatmulPerfMode.DoubleRow)

    with tile.TileContext(nc) as tc:
        from contextlib import ExitStack
        with ExitStack() as ctx:
            wp = ctx.enter_context(tc.tile_pool(name="persist", bufs=1))
            trp = ctx.enter_context(tc.tile_pool(name="trans", bufs=3))
            up = ctx.enter_context(tc.tile_pool(name="upool", bufs=6))
            obp = ctx.enter_context(tc.tile_pool(name="outbuf", bufs=2))
            smp = ctx.enter_context(tc.tile_pool(name="small", bufs=3))
            ksp = ctx.enter_context(tc.tile_pool(name="ksq", bufs=2))
            ppA = ctx.enter_context(tc.tile_pool(name="psA", bufs=3, space="PSUM"))
            ppS = ctx.enter_context(tc.tile_pool(name="psS", bufs=2, space="PSUM"))

            # ---------------- persistent SBUF ----------------
            kw1_sb = wp.tile([128, 8, 6, 2, 128], fp8, tag="kw1")
            nc.sync.dma_start(out=kw1_sb[:], in_=kw1_d[:])
            kb1n_sb = wp.tile([128, 8], f32, tag="kb1n")
            nc.sync.dma_start(out=kb1n_sb[:], in_=kb1n_d[:])
            kb1p_sb = wp.tile([128, 8], f32, tag="kb1p")
            nc.sync.dma_start(out=kb1p_sb[:], in_=kb1p_d[:])
            kw2_sb = wp.tile([128, 4, 2, N_ATT], fp8, tag="kw2")
            nc.sync.dma_start(out=kw2_sb[:], in_=kw2_d[:])
            kb2_sb = wp.tile([N_ATT, 1], f32, tag="kb2")
            nc.sync.dma_start(out=kb2_sb[:], in_=kb2_d[:])
            qw1adr_sb = wp.tile([N_MEL, 2, 128], fp8, tag="qw1adr")
            nc.sync.dma_start(out=qw1adr_sb[:], in_=qw1adr_d[:])
            qw1as_sb = wp.tile([N_MEL, 128], fp8, tag="qw1as")
            nc.sync.dma_start(out=qw1as_sb[:], in_=qw1as_d[:])
            qw1bdr_sb = wp.tile([N_MEL, 2, 32], fp8, tag="qw1bdr")
            nc.sync.dma_start(out=qw1bdr_sb[:], in_=qw1bdr_d[:])
            qw1bs_sb = wp.tile([N_MEL, 32], fp8, tag="qw1bs")
            nc.sync.dma_start(out=qw1bs_sb[:], in_=qw1bs_d[:])
            qb1an_sb = wp.tile([128, 1], f32, tag="qb1an")
            nc.sync.dma_start(out=qb1an_sb[:], in_=qb1an_d[:])
            qb1ap_sb = wp.tile([128, 1], f32, tag="qb1ap")
            nc.sync.dma_start(out=qb1ap_sb[:], in_=qb1ap_d[:])
            qb1bn_sb = wp.tile([32, 1], f32, tag="qb1bn")
            nc.sync.dma_start(out=qb1bn_sb[:], in_=qb1bn_d[:])
            qb1bp_sb = wp.tile([32, 1], f32, tag="qb1bp")
            nc.sync.dma_start(out=qb1bp_sb[:], in_=qb1bp_d[:])
            qw2a_sb = wp.tile([128, N_MEL], fp8, tag="qw2a")
            nc.sync.dma_start(out=qw2a_sb[:], in_=qw2a_d[:])
            qw2b_sb = wp.tile([32, N_MEL], fp8, tag="qw2b")
            nc.sync.dma_start(out=qw2b_sb[:], in_=qw2b_d[:])
            qb2_sb = wp.tile([N_MEL, 1], f32, tag="qb2")
            nc.sync.dma_start(out=qb2_sb[:], in_=qb2_d[:])
            qw3_sb = wp.tile([N_MEL, N_ATT], fp8, tag="qw3")
            nc.sync.dma_start(out=qw3_sb[:], in_=qw3_d[:])
            qb3_sb = wp.tile([N_ATT, 1], f32, tag="qb3")
            nc.sync.dma_start(out=qb3_sb[:], in_=qb3_d[:])

            mrow = wp.tile([128, B_LOC, T2], bf16, tag="mrow")
            for b in range(B_LOC):
                mb = mrowf_d[b]
                nc.gpsimd.dma_start(
                    out=mrow[:, b, :],
                    in_=bass.AP(tensor=mb.tensor, offset=mb.offset,
                                ap=[[0, 128]] + [list(d) for d in mb.ap]))

            nh_col = wp.tile([N_ATT, 1], bf16, tag="nh")
            nc.gpsimd.memset(nh_col[:], -0.5)

            # big batch-combined tensors
            kin = wp.tile([128, B_LOC, 4, T2 + 2], fp8, tag="kin")
            qin = wp.tile([N_MEL, B_LOC, T1 + 2], fp8, tag="qin")
            k1 = wp.tile([128, B_LOC, 8, T2], fp8, tag="k1")
            q1a = wp.tile([128, B_LOC, T1], fp8, tag="q1a")
            q1b = wp.tile([32, B_LOC, T1], fp8, tag="q1b")
            q2f = wp.tile([N_MEL, B_LOC, T1], fp8, tag="q2f")
            kaug = wp.tile([NAUG, B_LOC, T2], bf16, tag="kaug")
            qaug = wp.tile([NAUG, B_LOC, T1], bf16, tag="qaug")
            qsq = wp.tile([N_ATT, B_LOC, T1], bf16, tag="qsq")

            # augmented ones row + zero filler rows (broadcast DMAs)
            nc.sync.dma_start(out=qaug[96:97, :, :], in_=ones1_d[:])
            z1 = zeros1_d[0]
            nc.sync.dma_start(
                out=qaug[80:96, :, :],
                in_=bass.AP(tensor=z1.tensor, offset=z1.offset,
                            ap=[[0, 16], [1, B_LOC * T1]]))
            nc.sync.dma_start(
                out=kaug[80:96, :, :],
                in_=bass.AP(tensor=z1.tensor, offset=z1.offset,
                            ap=[[0, 16], [1, B_LOC * T2]]))

            # input loads
            for b in range(B_LOC):
                kb = keys8_d[b]
                nc.sync.dma_start(
                    out=kin[:, b, :, 1 : T2 + 1],
                    in_=dram_ap(kb, 0, [[T2, 128], [128 * T2, 4], [1, T2]]))
                nc.sync.dma_start(out=qin[:, b, 1 : T1 + 1], in_=qrs8_d[b])
            nc.gpsimd.memset(kin[:, :, :, 0:1], 0.0)
            nc.gpsimd.memset(kin[:, :, :, T2 + 1 : T2 + 2], 0.0)
            nc.gpsimd.memset(qin[:, :, 0:1], 0.0)
            nc.gpsimd.memset(qin[:, :, T1 + 1 : T1 + 2], 0.0)

            # kconv1 DR pair (rhs): k-tile t=(c,d) at free offset c*402+d in a
            # batch's [4, 402] block; pairs (2i, 2i+1)
            PAIR_OFF = []
            for i in range(6):
                t0, t1_ = 2 * i, 2 * i + 1
                c0, d0 = t0 // 3, t0 % 3
                o0 = c0 * (T2 + 2) + d0
                o1_ = (t1_ // 3) * (T2 + 2) + (t1_ % 3)
                PAIR_OFF.append((c0, d0, o1_ - o0))

            def kin_pair_ap(b, i):
                c0, d0, stride = PAIR_OFF[i]
                base = kin[:, b, c0, d0 : d0 + 1]
                return reap(base, [[stride, 2], [1, T2]])

            # ================= conv stages =================
            def kconv_stage(bp):  # bp = (b0, b1)
                b0, b1 = bp
                # conv1: 512->1024 k3, weight-stationary over the pair
                for m in range(8):
                    ps = ppA.tile([128, 2, 512], f32, tag="psA")
                    for i in range(6):
                        for jb, b in enumerate(bp):
                            mm_dr(ps[:, jb, 0:T2], kw1_sb[:, m, i],
                                  kin_pair_ap(b, i), start=(i == 0), stop=(i == 5),
                                  site="k1")
                    # evac both batches in one DVE op: relu(x + 16b) = 16*k1
                    # via max(x, -16b) + 16b (2-op tensor_scalar)
                    nc.vector.tensor_scalar(
                        out=reap(k1[:, b0, m, 0:1], [[8 * T2, 2], [1, T2]]),
                        in0=ps[:, :, 0:T2],
                        scalar1=kb1n_sb[:, m : m + 1], scalar2=kb1p_sb[:, m : m + 1],
                        op0=ALU.max, op1=ALU.add)
                # conv2: 1024->80 k1 (DR over 4 m-pairs)
                ps = ppA.tile([128, 2, 512], f32, tag="psA")
                for p in range(4):
                    for jb, b in enumerate(bp):
                        mm_dr(ps[0:N_ATT, jb, 0:T2], kw2_sb[:, p],
                              k1[:, b, 2 * p : 2 * p + 2, :],
                              start=(p == 0), stop=(p == 3), site="k2")
                nc.vector.tensor_scalar(
                    out=reap(kaug[0:N_ATT, b0, 0:1], [[T2, 2], [1, T2]]),
                    in0=ps[0:N_ATT, :, 0:T2],
                    scalar1=1.0 / (WSCALE * WSCALE), scalar2=kb2_sb[:],
                    op0=ALU.mult, op1=ALU.add)
                # k2 row: ksq = kaug^2 ; -0.5*k2 via matmul with -0.5 column
                psr = ppA.tile([128, 2, 512], f32, tag="psA")
                for jb, b in enumerate(bp):
                    ksq = ksp.tile([N_ATT, T2], bf16, tag="ksq")
                    sqeng = nc.vector if "sq_dve" in opts else nc.gpsimd
                    sqeng.tensor_tensor(
                        out=ksq[:], in0=kaug[0:N_ATT, b, :], in1=kaug[0:N_ATT, b, :],
                        op=ALU.mult)
                    nc.tensor.matmul(psr[0:1, jb, 0:T2], nh_col[:], ksq[:],
                                     start=True, stop=True)
                nc.scalar.activation(
                    out=reap(kaug[96:97, b0, 0:1], [[T2, 2], [1, T2]]),
                    in_=psr[0:1, :, 0:T2], func=AF.Copy, bias=0.0, scale=1.0)

            def qconv_stage(bp):
                b0, b1 = bp
                # conv1: 80->160 k3. DR over taps (0,1) + single tap 2.
                for pc in range(4):
                    o = pc * T2
                    ps_a = ppA.tile([128, 2, 512], f32, tag="psA")
                    for jb, b in enumerate(bp):
                        base = qin[:, b, o : o + 1]
                        mm_dr(ps_a[:, jb, 0:T2], qw1adr_sb[:],
                              reap(base, [[1, 2], [1, T2]]), start=True, stop=False,
                              site="q1")
                        nc.tensor.matmul(ps_a[:, jb, 0:T2], qw1as_sb[:],
                                         qin[:, b, o + 2 : o + 2 + T2],
                                         start=False, stop=True)
                    nc.vector.tensor_scalar(
                        out=reap(q1a[:, b0, o : o + 1], [[T1, 2], [1, T2]]),
                        in0=ps_a[:, :, 0:T2],
                        scalar1=qb1an_sb[:], scalar2=qb1ap_sb[:],
                        op0=ALU.max, op1=ALU.add)
                    ps_b = ppA.tile([128, 2, 512], f32, tag="psA")
                    for jb, b in enumerate(bp):
                        base = qin[:, b, o : o + 1]
                        mm_dr(ps_b[0:32, jb, 0:T2], qw1bdr_sb[:],
                              reap(base, [[1, 2], [1, T2]]), start=True, stop=False,
                              site="q1")
                        nc.tensor.matmul(ps_b[0:32, jb, 0:T2], qw1bs_sb[:],
                                         qin[:, b, o + 2 : o + 2 + T2],
                                         start=False, stop=True)
                    nc.vector.tensor_scalar(
                        out=reap(q1b[:, b0, o : o + 1], [[T1, 2], [1, T2]]),
                        in0=ps_b[0:32, :, 0:T2],
                        scalar1=qb1bn_sb[:], scalar2=qb1bp_sb[:],
                        op0=ALU.max, op1=ALU.add)
                # conv2: 160->80 relu
                for pc in range(4):
                    o = pc * T2
                    ps = ppA.tile([128, 2, 512], f32, tag="psA")
                    for jb, b in enumerate(bp):
                        nc.tensor.matmul(ps[0:N_MEL, jb, 0:T2], qw2a_sb[:],
                                         q1a[:, b, o : o + T2], start=True, stop=False)
                        nc.tensor.matmul(ps[0:N_MEL, jb, 0:T2], qw2b_sb[:],
                                         q1b[:, b, o : o + T2], start=False, stop=True)
                    # 3-op need (scale, bias, relu) -> ACT: 16*q2 = relu(ps/16+16b)
                    nc.scalar.activation(
                        out=reap(q2f[:, b0, o : o + 1], [[T1, 2], [1, T2]]),
                        in_=ps[0:N_MEL, :, 0:T2], func=AF.Relu,
                        bias=qb2_sb[:], scale=1.0 / WSCALE)
                # conv3: 80->80 (copy+bias)
                for pc in range(4):
                    o = pc * T2
                    ps = ppA.tile([128, 2, 512], f32, tag="psA")
                    for jb, b in enumerate(bp):
                        nc.tensor.matmul(ps[0:N_ATT, jb, 0:T2], qw3_sb[:],
                                         q2f[:, b, o : o + T2], start=True, stop=True)
                    nc.vector.tensor_scalar(
                        out=reap(qaug[0:N_ATT, b0, o : o + 1], [[T1, 2], [1, T2]]),
                        in0=ps[0:N_ATT, :, 0:T2],
                        scalar1=1.0 / (WSCALE * WSCALE), scalar2=qb3_sb[:],
                        op0=ALU.mult, op1=ALU.add)
                # squares for q2 reduction
                sqeng = nc.vector if "sq_dve" in opts else nc.gpsimd
                for b in bp:
                    sqeng.tensor_tensor(
                        out=qsq[:, b, :], in0=qaug[0:N_ATT, b, :],
                        in1=qaug[0:N_ATT, b, :], op=ALU.mult)

            # ================= score =================
            def score_group(b, g):
                o1, nj, _ = GROUPS[g]
                pmg = trp.tile([128, 4, T2], bf16, tag="pmg")
                pmb = pm_d[b]
                if nj == 4:
                    nc.sync.dma_start(
                        out=pmg[:, 0:nj, :],
                        in_=dram_ap(pmb, o1 * T2, [[T2, 128], [128 * T2, nj], [1, T2]]))
                else:
                    tp0 = min(128, T1 - o1)
                    nc.sync.dma_start(
                        out=pmg[:tp0, 0, :],
                        in_=dram_ap(pmb, o1 * T2, [[T2, tp0], [1, T2]]))
                ab = obp.tile([128, 4, T2], bf16, tag="attnb")
                lb = obp.tile([128, 4, T2], bf16, tag="logpb")
                safe = "safe_small" in opts
                if safe:
                    S1s = [smp.tile([128, 1], f32, tag=f"S1_{jj}", name=f"S1_{b}_{g}_{jj}") for jj in range(nj)]
                    rS1s = [smp.tile([128, 1], f32, tag=f"rS1_{jj}", name=f"rS1_{b}_{g}_{jj}") for jj in range(nj)]
                    S2s = [smp.tile([128, 1], f32, tag=f"S2_{jj}", name=f"S2_{b}_{g}_{jj}") for jj in range(nj)]
                    rS2s = [smp.tile([128, 1], f32, tag=f"rS2_{jj}", name=f"rS2_{b}_{g}_{jj}") for jj in range(nj)]
                    q2ns = [smp.tile([128, 1], f32, tag=f"q2n_{jj}", name=f"q2n_{b}_{g}_{jj}") for jj in range(nj)]
                else:
                    S1g = smp.tile([128, 4], f32, tag="S1g")
                    rS1g = smp.tile([128, 4], f32, tag="rS1g")
                    S2g = smp.tile([128, 4], f32, tag="S2g")
                    rS2g = smp.tile([128, 4], f32, tag="rS2g")
                    q2ng = smp.tile([128, 4], f32, tag="q2ng")
                us, ums, tps = [], [], []
                for jj in range(nj):
                    t0 = o1 + jj * 128
                    tp = min(128, T1 - t0)
                    tps.append(tp)
                    pss = ppS.tile([128, 512], f32, tag="pss")
                    nc.tensor.matmul(pss[:tp, 0:T2], qaug[:, b, t0 : t0 + tp],
                                     kaug[:, b, :], start=True, stop=True)
                    nc.tensor.matmul(pss[:tp, T2 : T2 + 1], qsq[:, b, t0 : t0 + tp],
                                     nh_col[:], start=True, stop=True)
                    q2n_ap = q2ns[jj][:tp] if safe else q2ng[:tp, jj : jj + 1]
                    S1_ap = S1s[jj][:tp] if safe else S1g[:tp, jj : jj + 1]
                    S2_ap = S2s[jj][:tp] if safe else S2g[:tp, jj : jj + 1]
                    nc.vector.tensor_scalar(
                        out=q2n_ap, in0=pss[:tp, T2 : T2 + 1],
                        scalar1=0.001, scalar2=None, op0=ALU.mult)
                    w_t = trp.tile([128, T2], bf16, tag="w")
                    nc.scalar.activation(
                        out=w_t[:tp], in_=pss[:tp, 0:T2], func=AF.Exp,
                        bias=q2n_ap, scale=0.001, accum_out=S1_ap)
                    u_t = up.tile([128, T2], bf16, tag="u")
                    ueng = nc.vector if "u_dve" in opts else nc.gpsimd
                    ueng.tensor_tensor(out=u_t[:tp], in0=w_t[:tp],
                                       in1=pmg[:tp, jj, :], op=ALU.mult)
                    um_t = up.tile([128, T2], bf16, tag="um")
                    if "um_stt" in opts:
                        nc.vector.scalar_tensor_tensor(
                            out=um_t[:tp], in0=u_t[:tp], scalar=1.0,
                            in1=mrow[:tp, b, :], op0=ALU.mult, op1=ALU.mult,
                            accum_out=S2_ap)
                    else:
                        nc.vector.tensor_tensor_reduce(
                            out=um_t[:tp], in0=u_t[:tp], in1=mrow[:tp, b, :],
                            scale=1.0, scalar=0.0, op0=ALU.mult, op1=ALU.add,
                            accum_out=S2_ap)
                    if safe:
                        nc.vector.reciprocal(rS1s[jj][:tp], S1s[jj][:tp])
                        nc.vector.reciprocal(rS2s[jj][:tp], S2s[jj][:tp])
                    us.append(u_t)
                    ums.append(um_t)
                if not safe:
                    tpm = max(tps)
                    nc.vector.reciprocal(rS1g[:tpm, 0:nj], S1g[:tpm, 0:nj])
                    nc.vector.reciprocal(rS2g[:tpm, 0:nj], S2g[:tpm, 0:nj])
                for jj in range(nj):
                    tp = tps[jj]
                    rS1_ap = rS1s[jj][:tp] if safe else rS1g[:tp, jj : jj + 1]
                    rS2_ap = rS2s[jj][:tp] if safe else rS2g[:tp, jj : jj + 1]
                    nc.scalar.activation(
                        out=lb[:tp, jj, :], in_=us[jj][:tp], func=AF.Ln,
                        bias=0.0, scale=rS1_ap)
                    aeng = nc.vector if "attn_dve" in opts else nc.gpsimd
                    aeng.tensor_scalar(
                        out=ab[:tp, jj, :], in0=ums[jj][:tp],
                        scalar1=rS2_ap, scalar2=None, op0=ALU.mult)
                if nj == 4:
                    for dd, buf in ((attn_d, ab), (logp_d, lb)):
                        db = dd[b]
                        nc.sync.dma_start(
                            out=dram_ap(db, o1 * T2, [[T2, 128], [128 * T2, nj], [1, T2]]),
                            in_=buf[:, 0:nj, :])
                else:
                    tp = tps[0]
                    for dd, buf in ((attn_d, ab), (logp_d, lb)):
                        db = dd[b]
                        nc.sync.dma_start(
                            out=dram_ap(db, o1 * T2, [[T2, tp], [1, T2]]),
                            in_=buf[:tp, 0, :])

            # ================= schedule =================
            do_k = "no_kconv" not in opts
            do_q = "no_qconv" not in opts
            do_s = "no_score" not in opts
            if do_k:
                kconv_stage((0, 1))
            if do_q:
                qconv_stage((0, 1))
            # score b0/b1 interleaved with conv of batches 2,3
            if do_s:
                score_group(0, 0)
            if do_k:
                kconv_stage((2, 3))
            if do_s:
                score_group(0, 1)
                score_group(0, 2)
            if do_q:
                qconv_stage((2, 3))
            if do_s:
                score_group(0, 3)
                for bb in range(1, 4):
                    for g in range(4):
                        score_group(bb, g)

    # Pin ONE activation table set covering Exp/Ln/Relu/Copy so the
    # table-load pass hoists a single ACT_TABLE_LOAD.
    import concourse.bacc as bacc_mod
    _orig_tabs = bacc_mod.get_activation_tables
    def _pinned_tabs(arch):
        tabs = _orig_tabs(arch)
        used = {AF.Exp, AF.Ln, AF.Relu, AF.Copy}
        out = {}
        for name, fns in tabs.items():
            if name == "natural_log_exp_and_others":
                out[name] = set(fns)
            else:
                out[name] = set(fns) - used
        return out
    bacc_mod.get_activation_tables = _pinned_tabs
    try:
        nc.compile()
    finally:
        bacc_mod.get_activation_tables = _orig_tabs
    return nc


def _prep_weights(kw1, kb1, kw2, kb2, qw1, qb1, qw2, qb2, qw3, qb3):
    s = WSCALE
    # kw1 [1024, 512, 3] -> [ci128, m, pair, j, co] with ktile t=(c*3+d)
    t = (s * kw1).reshape(8, 128, 4, 128, 3)  # [m, co, c, ci, d]
    t = t.transpose(3, 0, 2, 4, 1)            # [ci, m, c, d, co]
    kw1p = np.ascontiguousarray(
        t.reshape(128, 8, 12, 128)[:, :, :, :].reshape(128, 8, 6, 2, 128)).astype(FP8)
    kb1t = np.ascontiguousarray(kb1.reshape(8, 128).T).astype(np.float32)
    # kw2 [80, 1024, 1] -> [ci128, pair, j, co]
    t = (s * kw2[:, :, 0]).reshape(N_ATT, 8, 128)  # [co, m, ci]
    kw2p = np.ascontiguousarray(
        t.transpose(2, 1, 0).reshape(128, 4, 2, N_ATT)).astype(FP8)
    kb2c = np.ascontiguousarray(kb2[:, None]).astype(np.float32)
    # qw1 [160, 80, 3] -> dr: [ci, d(0,1), co]; s: [ci, co] (d=2)
    t = (s * qw1).transpose(1, 2, 0)  # [ci, d, co]
    qw1adr = np.ascontiguousarray(t[:, 0:2, 0:128]).astype(FP8)
    qw1as = np.ascontiguousarray(t[:, 2, 0:128]).astype(FP8)
    qw1bdr = np.ascontiguousarray(t[:, 0:2, 128:160]).astype(FP8)
    qw1bs = np.ascontiguousarray(t[:, 2, 128:160]).astype(FP8)
    qb1a16 = (s * qb1[:128, None]).astype(np.float32)
    qb1b16 = (s * qb1[128:, None]).astype(np.float32)
    qw2a = np.ascontiguousarray((s * qw2)[:, :128, 0].T).astype(FP8)
    qw2b = np.ascontiguousarray((s * qw2)[:, 128:, 0].T).astype(FP8)
    qb2c = np.ascontiguousarray(s * qb2[:, None]).astype(np.float32)  # 16*qb2
    qw3T = np.ascontiguousarray((s * qw3)[:, :, 0].T).astype(FP8)
    qb3c = np.ascontiguousarray(qb3[:, None]).astype(np.float32)
    return dict(kw1p=kw1p, kb1n=np.ascontiguousarray(-s * kb1t),
                kb1p=np.ascontiguousarray(s * kb1t), kw2p=kw2p, kb2c=kb2c,
                qw1adr=qw1adr, qw1as=qw1as, qw1bdr=qw1bdr, qw1bs=qw1bs,
                qb1an=np.ascontiguousarray(-qb1a16),
                qb1ap=np.ascontiguousarray(qb1a16),
                qb1bn=np.ascontiguousarray(-qb1b16),
                qb1bp=np.ascontiguousarray(qb1b16),
                qw2a=qw2a, qw2b=qw2b, qb2c=qb2c, qw3T=qw3T, qb3c=qb3c)


def kernel(queries, keys, mask, attn_prior,
           kw1, kb1, kw2, kb2, qw1, qb1, qw2, qb2, qw3, qb3):
    from concourse.bass_utils import run_bass_kernel_spmd

    if "nc" not in _CACHE:
        _CACHE["nc"] = _build_program()
    nc = _CACHE["nc"]

    queries8 = np.asarray(queries, dtype=np.float32).astype(FP8)
    keys8 = np.asarray(keys, dtype=np.float32).astype(FP8)
    pm = (np.asarray(attn_prior, dtype=np.float32) + 1e-8).astype(BF16)
    mrowf = (~np.asarray(mask)).astype(np.float32).astype(BF16)  # 1=keep
    ones1 = np.ones((1, B_LOC * T1), dtype=BF16)
    zeros1 = np.zeros((1, B_LOC * T1), dtype=BF16)
    w = _prep_weights(np.asarray(kw1), np.asarray(kb1), np.asarray(kw2),
                      np.asarray(kb2), np.asarray(qw1), np.asarray(qb1),
                      np.asarray(qw2), np.asarray(qb2), np.asarray(qw3),
                      np.asarray(qb3))

    in_maps = []
    for c in range(N_CORES):
        sl = slice(B_LOC * c, B_LOC * (c + 1))
        m = {
            "qrs8": np.ascontiguousarray(queries8[sl]),
            "keys8": np.ascontiguousarray(keys8[sl]),
            "pm": np.ascontiguousarray(pm[sl]),
            "mrowf": np.ascontiguousarray(mrowf[sl]),
            "ones1": ones1,
            "zeros1": zeros1,
        }
        m.update(w)
        in_maps.append(m)

    res = run_bass_kernel_spmd(nc, in_maps, core_ids=list(range(N_CORES)),
                               **_CACHE.get("run_kwargs", {}))
    _CACHE["last_result"] = res

    attn = np.empty((B, 1, T1, T2), np.float32)
    logp = np.empty((B, 1, T1, T2), np.float32)
    for c in range(N_CORES):
        attn[B_LOC * c : B_LOC * (c + 1), 0] = res.results[c]["attn"].astype(np.float32)
        logp[B_LOC * c : B_LOC * (c + 1), 0] = res.results[c]["logp"].astype(np.float32)
    return attn, logp

